# revision 17
# baseline (speedup 1.0000x reference)
"""Trainium2 Bass kernel for nn_Attention_91225105367483.

Spatial attention: x[B=2,T=8,H=32,W=32,D=768] -> 16 frames x 1024 tokens.
Data-parallel over frames: 8 cores x 2 frames each. No collectives.

v2 schedule (vs the 477us baseline):
  - Attention is ACT(exp)-bound: 192 exps x ~1.1us = 213us/core. All other
    engines' work is arranged to hide under it.
  - Frame 1's qkv+rope is EMITTED interleaved with frame 0's attention
    units, with its own PSUM tag ring, so PE fills exp-wait gaps (the
    baseline serialized qkv(f1) after attention(f0) via the shared "mm"
    psum ring).
  - Softmax normalize: reciprocal straight off the PSUM denominator row,
    GPSIMD partition_broadcast, one DVE multiply (3 ops, was 6 + a PE
    broadcast matmul). Frees ~85us of DVE and ~10us of PE.
  - qkv psum->sbuf casts for frame 0 run on the idle Scalar engine.
  - Const/x DMAs issue from two queues (sync + scalar) with x chunk 0 and
    wqkv first: first matmul at ~10us instead of 26.5us.
  - Out-proj interleaves under frame 1's attention; out DMAs issue from
    the gpsimd queue.

Per-core layout (hardcoded): xT [768, 2048] d-major bf16; q,k TRANSPOSED
[64hd, seq] per head-pair so attention needs no transposes; v natural
[seq, 64+1] with a ones column emitting softmax denominators from the
same attn@v stream; scores transposed [keys, queries], exp on ACT with
the 1/8 scale folded in; softmax skips max-subtraction (scores are O(1),
exp exact to 2ULP).
"""
import sys

sys.path.insert(0, "/opt/trn_rl_repo")

import numpy as np
import ml_dtypes

BF16 = ml_dtypes.bfloat16

B, T, D = 2, 8, 768
NH, HD = 12, 64
NCORES = 8
FPC = 2  # frames per core

_built = {}

import os
F_NO_INTERLEAVE = os.environ.get("KV2_NO_INTERLEAVE", "0") == "1"
F_OLD_NORM = os.environ.get("KV2_OLD_NORM", "0") == "1"
F_NO_ACT_CAST = os.environ.get("KV2_NO_ACT_CAST", "0") == "1"
F_SYNC_DMA = os.environ.get("KV2_SYNC_DMA", "0") == "1"



def _host_rope(H, W, head_dim):
    """Replicates reference._rope_cos_sin in numpy fp32."""
    half = head_dim // 4
    inv_freq = (1.0 / (10000.0 ** (np.arange(half, dtype=np.float32) / half))).astype(
        np.float32
    )
    th_h = np.arange(H, dtype=np.float32)[:, None] * inv_freq  # [H, half]
    th_w = np.arange(W, dtype=np.float32)[:, None] * inv_freq  # [W, half]
    cos = np.concatenate(
        [
            np.broadcast_to(np.cos(th_h)[:, None, :], (H, W, half)),
            np.broadcast_to(np.cos(th_w)[None, :, :], (H, W, half)),
        ],
        axis=-1,
    )
    sin = np.concatenate(
        [
            np.broadcast_to(np.sin(th_h)[:, None, :], (H, W, half)),
            np.broadcast_to(np.sin(th_w)[None, :, :], (H, W, half)),
        ],
        axis=-1,
    )
    cos = np.repeat(cos, 2, axis=-1).reshape(H * W, head_dim).astype(np.float32)
    sin = np.repeat(sin, 2, axis=-1).reshape(H * W, head_dim).astype(np.float32)
    return cos, sin


def _rot_matT():
    """RT = R.T where (R @ q)[2i] = -q[2i+1], (R @ q)[2i+1] = q[2i]."""
    RT = np.zeros((128, 128), dtype=np.float32)
    for i in range(64):
        RT[2 * i + 1, 2 * i] = -1.0
        RT[2 * i, 2 * i + 1] = 1.0
    return RT


def build_nc(H, W):
    """Builds the per-core Bass program. S = H*W tokens per frame."""
    import concourse.bass as bass
    import concourse.tile as tile
    from concourse import bacc, mybir, library_config

    dt = mybir.dt
    S = H * W
    SL = FPC * S  # tokens per core
    QCS = min(512, S)  # query-chunk size
    NQC = S // QCS  # query chunks per frame
    KT = S // 128  # key tiles per frame
    SC = min(512, S)  # token chunk for qkv proj
    NSCF = S // SC  # qkv token chunks per frame
    NSTF = S // 128  # s-tiles per frame
    NDC = D // 128  # 6 d-chunks
    NPAIR = NH // 2  # 6 head pairs
    SPW = 2 * QCS  # scores tile width: 2 heads interleaved

    nc = bacc.Bacc("TRN2", target_bir_lowering=False, debug=False)

    xT = nc.dram_tensor("xT", [D, SL], dt.bfloat16, kind="ExternalInput")
    wqkvT = nc.dram_tensor("wqkvT", [D, 3 * D], dt.bfloat16, kind="ExternalInput")
    w_outT = nc.dram_tensor("w_outT", [D, D], dt.bfloat16, kind="ExternalInput")
    cosP = nc.dram_tensor("cosP", [128, SL], dt.bfloat16, kind="ExternalInput")
    sinP = nc.dram_tensor("sinP", [128, SL], dt.bfloat16, kind="ExternalInput")
    rotT = nc.dram_tensor("rotT", [128, 128], dt.bfloat16, kind="ExternalInput")
    bias_rep = nc.dram_tensor("bias_rep", [128, D], dt.float32, kind="ExternalInput")
    out = nc.dram_tensor("out", [SL, D], dt.float32, kind="ExternalOutput")

    ActF = mybir.ActivationFunctionType
    scale = 1.0 / np.sqrt(HD)

    with tile.TileContext(nc) as tc:
        import contextlib

        ctx = contextlib.ExitStack()
        with ctx:
            const = ctx.enter_context(tc.tile_pool(name="const", bufs=1))
            xt_pool = ctx.enter_context(tc.tile_pool(name="xt", bufs=14))
            qk_pool = ctx.enter_context(tc.tile_pool(name="qk", bufs=1))
            v_pool = ctx.enter_context(tc.tile_pool(name="v", bufs=1))
            ot_pool = ctx.enter_context(tc.tile_pool(name="ot", bufs=1))
            et_pool = ctx.enter_context(tc.tile_pool(name="et", bufs=7))
            rt_pool = ctx.enter_context(tc.tile_pool(name="rt", bufs=3))
            rc_pool = ctx.enter_context(tc.tile_pool(name="rc", bufs=3))
            bct_pool = ctx.enter_context(tc.tile_pool(name="bct", bufs=3))
            outsb_pool = ctx.enter_context(tc.tile_pool(name="outsb", bufs=2))
            sc_ps = ctx.enter_context(tc.tile_pool(name="scps", bufs=2, space="PSUM"))
            av_ps = ctx.enter_context(tc.tile_pool(name="avps", bufs=2, space="PSUM"))
            mm_ps = ctx.enter_context(tc.tile_pool(name="mmps", bufs=2, space="PSUM"))

            # ---- constants + x prefetch (two issue queues, critical first) ----
            xts = [[None] * NDC for _ in range(FPC * NSCF)]
            for gc in range(FPC * NSCF):
                for d in range(NDC):
                    xts[gc][d] = xt_pool.tile(
                        [128, SC], dt.bfloat16, tag="xt", name=f"xt_{gc}_{d}"
                    )
            w_t = []
            for d in range(NDC):
                w_t.append(
                    const.tile([128, 3 * D], dt.bfloat16, tag=f"wqkv{d}", name=f"wqkv{d}")
                )
            wo_t = []
            bias_t = None
            # sync queue: x chunk 0, then wqkv, then the rest of x
            for d in range(NDC):
                nc.sync.dma_start(xts[0][d][:], xT[d * 128 : (d + 1) * 128, 0:SC])
            for d in range(NDC):
                weng = nc.gpsimd if d % 2 == 0 else nc.scalar
                weng.dma_start(w_t[d][:], wqkvT[d * 128 : (d + 1) * 128, :])
            for gc in range(1, FPC * NSCF):
                for d in range(NDC):
                    nc.sync.dma_start(
                        xts[gc][d][:], xT[d * 128 : (d + 1) * 128, gc * SC : (gc + 1) * SC]
                    )
            # scalar queue: rope tables, then out-proj weights (needed late)
            eng2 = nc.sync if F_SYNC_DMA else nc.scalar
            cos_t = const.tile([128, SL], dt.bfloat16, tag="cos")
            eng2.dma_start(cos_t[:], cosP[:])
            sin_t = const.tile([128, SL], dt.bfloat16, tag="sin")
            eng2.dma_start(sin_t[:], sinP[:])
            rot_t = const.tile([128, 128], dt.bfloat16, tag="rot")
            eng2.dma_start(rot_t[:], rotT[:])
            for d in range(NDC):
                t = const.tile([128, D], dt.bfloat16, tag=f"wout{d}", name=f"wout{d}")
                eng2.dma_start(t[:], w_outT[d * 128 : (d + 1) * 128, :])
                wo_t.append(t)
            bias_t = const.tile([128, D], dt.float32, tag="bias")
            eng2.dma_start(bias_t[:], bias_rep[:])
            ones_t = const.tile([1, 64], dt.bfloat16, tag="ones")
            nc.vector.memset(ones_t[:], 1.0)
            nc.gpsimd.load_library(library_config.attn)

            # ---- per-frame activation tiles ----
            frames = {}

            def make_frame(f):
                fr = {}
                fr["qk_q"] = [
                    qk_pool.tile([128, S], dt.bfloat16, tag=f"q{p}", bufs=2, name=f"qkq{f}_{p}")
                    for p in range(NPAIR)
                ]
                fr["qk_k"] = [
                    qk_pool.tile([128, S], dt.bfloat16, tag=f"k{p}", bufs=2, name=f"qkk{f}_{p}")
                    for p in range(NPAIR)
                ]
                fr["v_sb"] = [
                    v_pool.tile([128, NH * 65], dt.bfloat16, tag=f"v{i}", bufs=2, name=f"vsb{f}_{i}")
                    for i in range(NSTF)
                ]
                for i in range(NSTF):
                    vv = fr["v_sb"][i][:].rearrange("p (h c) -> p h c", h=NH)
                    nc.vector.memset(vv[:, :, 64:65], 1.0)
                frames[f] = fr

            def make_frame_ot(f):
                frames[f]["ot_sb"] = [
                    ot_pool.tile([128, S], dt.bfloat16, tag=f"ot{d}", bufs=2, name=f"otsb{f}_{d}")
                    for d in range(NDC)
                ]

            # ---- emit helpers ----
            def emit_qk_chain(f, c, et):
                """One [128e, 512tok] output chain of the transposed q/k proj."""
                gc = f * NSCF + c
                ps = mm_ps.tile([128, SC], dt.float32, tag="mm", name=f"qkps_{f}_{c}_{et}")
                for d in range(NDC):
                    nc.tensor.matmul(
                        ps[:],
                        w_t[d][:, et * 128 : (et + 1) * 128],
                        xts[gc][d][:],
                        start=(d == 0),
                        stop=(d == NDC - 1),
                    )
                fr = frames[f]
                dst = fr["qk_q"][et] if et < NDC else fr["qk_k"][et - NDC]
                eng = nc.scalar if (f == 0 and not F_NO_ACT_CAST) else nc.vector
                if eng is nc.scalar:
                    nc.scalar.copy(dst[:, c * SC : (c + 1) * SC], ps[:])
                else:
                    nc.vector.tensor_copy(dst[:, c * SC : (c + 1) * SC], ps[:])

            def emit_rope(f, p, which, c):
                """RoPE chunk c for q or k of pair p: tens = tens*cos + rot(tens)*sin."""
                fr = frames[f]
                tens = fr["qk_q"][p] if which == "q" else fr["qk_k"][p]
                sl_ = slice(c * SC, (c + 1) * SC)
                rps = mm_ps.tile([128, SC], dt.float32, tag="mm", name=f"rps_{f}_{p}_{which}_{c}")
                nc.tensor.matmul(rps[:], rot_t[:], tens[:, sl_], start=True, stop=True)
                t1 = rt_pool.tile([128, SC], dt.bfloat16, tag="rt1", name=f"rt1_{f}_{p}_{which}_{c}")
                nc.vector.tensor_mul(t1[:], rps[:], sin_t[:, sl_])
                t2 = rt_pool.tile([128, SC], dt.bfloat16, tag="rt2", name=f"rt2_{f}_{p}_{which}_{c}")
                nc.vector.tensor_mul(t2[:], tens[:, sl_], cos_t[:, sl_])
                nc.vector.tensor_add(tens[:, sl_], t1[:], t2[:])

            def emit_v_chain(f, c, st, nch):
                """V projection for s-tile (c,st), e-columns [n0:n1)."""
                gc = f * NSCF + c
                lst = c * (SC // 128) + st
                n0, n1 = (0, 512) if nch == 0 else (512, D)
                h0, h1 = (0, 8) if nch == 0 else (8, NH)
                ps = mm_ps.tile([128, SC], dt.float32, tag="mm", name=f"vps_{f}_{lst}_{nch}")
                for d in range(NDC):
                    nc.tensor.matmul(
                        ps[:, : n1 - n0],
                        xts[gc][d][:, st * 128 : (st + 1) * 128],
                        w_t[d][:, 2 * D + n0 : 2 * D + n1],
                        start=(d == 0),
                        stop=(d == NDC - 1),
                    )
                vv = frames[f]["v_sb"][lst][:].rearrange("p (h c) -> p h c", h=NH)
                pv = ps[:, : n1 - n0].rearrange("p (h c) -> p h c", c=HD)
                if f == 0 and not F_NO_ACT_CAST:
                    nc.scalar.copy(vv[:, h0:h1, 0:HD], pv[:])
                else:
                    nc.vector.tensor_copy(vv[:, h0:h1, 0:HD], pv[:])

            def emit_att_unit(f, p, qc):
                """Attention for head-pair p, query chunk qc: scores+exp+av+norm."""
                fr = frames[f]
                qsl = slice(qc * QCS, (qc + 1) * QCS)
                avp = [
                    av_ps.tile([128, QCS], dt.float32, tag="av", name=f"avp_{f}_{p}_{qc}_{hh}")
                    for hh in range(2)
                ]
                LAG = 5
                ets = [None] * KT

                def emit_av(g):
                    for hh in range(2):
                        h = 2 * p + hh
                        nc.tensor.matmul(
                            avp[hh][0:65, :],
                            fr["v_sb"][g][:, h * 65 : h * 65 + 65],
                            ets[g][:, hh * QCS : (hh + 1) * QCS],
                            start=(g == 0),
                            stop=(g == KT - 1),
                        )

                for g in range(KT):
                    ksl = slice(g * 128, (g + 1) * 128)
                    sp = sc_ps.tile([128, SPW], dt.float32, tag="sc", name=f"sp_{f}_{p}_{qc}_{g}")
                    for hh in range(2):
                        rb = 64 * hh
                        nc.tensor.matmul(
                            sp[:, hh * QCS : (hh + 1) * QCS],
                            fr["qk_k"][p][rb : rb + 64, ksl],
                            fr["qk_q"][p][rb : rb + 64, qsl],
                            start=True,
                            stop=True,
                            tile_position=(rb, 0),
                        )
                    ets[g] = et_pool.tile([128, SPW], dt.bfloat16, tag="et", name=f"et_{f}_{p}_{qc}_{g}")
                    nc.scalar.activation(ets[g][:], sp[:], ActF.Exp, scale=float(scale))
                    if g >= LAG:
                        emit_av(g - LAG)
                for g in range(KT - LAG, KT):
                    emit_av(g)
                # normalize: o[0:64] * (1/r), r = row 64 of the psum accumulator
                for hh in range(2):
                    rb = 64 * hh
                    if F_OLD_NORM:
                        rr = rc_pool.tile([1, QCS], dt.float32, tag="rr", name=f"rr_{f}_{p}_{qc}_{hh}")
                        nc.vector.tensor_copy(rr[:], avp[hh][64:65, :])
                        ou = bct_pool.tile([64, QCS], dt.float32, tag="ou", name=f"ou_{f}_{p}_{qc}_{hh}")
                        nc.vector.tensor_copy(ou[:], avp[hh][0:64, :])
                        rc = rc_pool.tile([1, QCS], dt.float32, tag="rc", name=f"rc_{f}_{p}_{qc}_{hh}")
                        nc.vector.reciprocal_approx_fast(rc[:], rr[:])
                        rcb = rc_pool.tile([1, QCS], dt.bfloat16, tag="rcb", name=f"rcb_{f}_{p}_{qc}_{hh}")
                        nc.vector.tensor_copy(rcb[:], rc[:])
                        bc = mm_ps.tile([128, QCS], dt.float32, tag="mm", name=f"bc_{f}_{p}_{qc}_{hh}")
                        nc.tensor.matmul(bc[0:64, :], ones_t[:], rcb[:], start=True, stop=True)
                        nc.vector.tensor_mul(
                            fr["ot_sb"][p][rb : rb + 64, qsl], ou[:], bc[0:64, :]
                        )
                    else:
                        rr = rc_pool.tile([1, QCS], dt.float32, tag="rr", name=f"nrr_{f}_{p}_{qc}_{hh}")
                        nc.vector.tensor_copy(rr[:], avp[hh][64:65, :])
                        rc = rc_pool.tile([1, QCS], dt.float32, tag="rc", name=f"rc_{f}_{p}_{qc}_{hh}")
                        nc.vector.reciprocal_approx_fast(rc[:], rr[:])
                        bct = bct_pool.tile([64, QCS], dt.float32, tag="bct", name=f"bct_{f}_{p}_{qc}_{hh}")
                        nc.gpsimd.partition_broadcast(bct[:], rc[:])
                        nc.vector.tensor_mul(
                            fr["ot_sb"][p][rb : rb + 64, qsl], avp[hh][0:64, :], bct[:]
                        )

            def emit_outproj(f, st):
                fr = frames[f]
                osb = outsb_pool.tile([128, D], dt.float32, tag="osb", name=f"osb_{f}_{st}")
                for nch in range(2):
                    n0, n1 = (0, 512) if nch == 0 else (512, D)
                    ps = mm_ps.tile([128, 512], dt.float32, tag="mm", name=f"ops_{f}_{st}_{nch}")
                    for d in range(NDC):
                        nc.tensor.matmul(
                            ps[:, : n1 - n0],
                            fr["ot_sb"][d][:, st * 128 : (st + 1) * 128],
                            wo_t[d][:, n0:n1],
                            start=(d == 0),
                            stop=(d == NDC - 1),
                        )
                    nc.vector.tensor_add(osb[:, n0:n1], ps[:, : n1 - n0], bias_t[:, n0:n1])
                (nc.sync if F_SYNC_DMA else nc.gpsimd).dma_start(
                    out[f * S + st * 128 : f * S + (st + 1) * 128, :], osb[:]
                )

            def qkv_items(f):
                """Closures producing frame f's qkv+rope in dependency-friendly
                order: per chunk, k-projections (then their rope) before q."""
                items = []
                for c in range(NSCF):
                    for et in list(range(NDC, 2 * NDC)) + list(range(NDC)):
                        p = et - NDC if et >= NDC else et
                        which = "k" if et >= NDC else "q"
                        items.append(
                            lambda f=f, c=c, et=et, p=p, w=which: (
                                emit_qk_chain(f, c, et),
                                emit_rope(f, p, w, c),
                            )
                        )
                    for st in range(SC // 128):
                        for nch in range(2):
                            items.append(
                                lambda f=f, c=c, st=st, nch=nch: emit_v_chain(f, c, st, nch)
                            )
                return items

            units = [(p, qc) for p in range(NPAIR) for qc in range(NQC)]

            # ---- P1: frame 0 qkv + rope (casts/copies on idle ACT) ----
            make_frame(0)
            for it in qkv_items(0):
                it()

            # ---- P2: frame 0 attention interleaved with frame 1 qkv ----
            make_frame(1)
            make_frame_ot(0)
            work = qkv_items(1)
            done = 0
            for i, (p, qc) in enumerate(units):
                emit_att_unit(0, p, qc)
                if F_NO_INTERLEAVE:
                    continue
                want = (i + 1) * len(work) // len(units)
                while done < want:
                    work[done]()
                    done += 1
            while done < len(work):
                work[done]()
                done += 1

            # ---- P3: frame 1 attention interleaved with frame 0 out-proj ----
            make_frame_ot(1)
            done = 0
            for i, (p, qc) in enumerate(units):
                emit_att_unit(1, p, qc)
                if F_NO_INTERLEAVE:
                    continue
                want = (i + 1) * NSTF // len(units)
                while done < want:
                    emit_outproj(0, done)
                    done += 1
            while done < NSTF:
                emit_outproj(0, done)
                done += 1

            # ---- P4: frame 1 out-proj ----
            for st in range(NSTF):
                emit_outproj(1, st)

    nc.compile()
    return nc


def _prep_inputs(x, w_qkv, w_out, b_out, H, W):
    """Host-side prep: shard + transpose + cast. Returns per-core in_maps."""
    S = H * W
    SL = FPC * S
    nframes = x.shape[0] * x.shape[1]
    ncores = nframes // FPC
    xf = np.asarray(x, dtype=np.float32).reshape(nframes, S, D)

    wqkvT = np.ascontiguousarray(np.asarray(w_qkv, np.float32).T).astype(BF16)
    w_outT = np.ascontiguousarray(np.asarray(w_out, np.float32).T).astype(BF16)
    cos, sin = _host_rope(H, W, HD)  # [S, 64]
    cosP = np.tile(cos.T, (2, FPC)).astype(BF16)  # [128, SL]
    sinP = np.tile(sin.T, (2, FPC)).astype(BF16)
    rotT = _rot_matT().astype(BF16)
    bias_rep = np.tile(np.asarray(b_out, np.float32)[None, :], (128, 1))

    in_maps = []
    for c in range(ncores):
        shard = xf[c * FPC : (c + 1) * FPC].reshape(SL, D)
        xT = np.ascontiguousarray(shard.T).astype(BF16)  # [768, SL]
        in_maps.append(
            dict(
                xT=xT,
                wqkvT=wqkvT,
                w_outT=w_outT,
                cosP=cosP,
                sinP=sinP,
                rotT=rotT,
                bias_rep=bias_rep,
            )
        )
    return in_maps


def run(x, w_qkv, w_out, b_out, trace=False, tmpdir=None):
    from concourse import bass_utils

    Hd, Wd = x.shape[2], x.shape[3]
    key = (Hd, Wd)
    if key not in _built:
        _built[key] = build_nc(Hd, Wd)
    nc = _built[key]
    in_maps = _prep_inputs(x, w_qkv, w_out, b_out, Hd, Wd)
    res = bass_utils.run_bass_kernel_spmd(
        nc, in_maps, core_ids=list(range(len(in_maps))), trace=trace, tmpdir=tmpdir
    )
    S = Hd * Wd
    outs = [r["out"] for r in res.results]
    full = np.concatenate(outs, axis=0).reshape(B, T, Hd, Wd, D).astype(np.float32)
    return full, res


def kernel(x, w_qkv, w_out, b_out):
    full, _ = run(x, w_qkv, w_out, b_out, trace=False)
    return full


# revision 19
# speedup vs baseline: 1.1933x; 1.1933x over previous
"""Trainium2 Bass kernel for nn_Attention_91225105367483.

Spatial attention: x[B=2,T=8,H=32,W=32,D=768] -> 16 frames x 1024 tokens.
Data-parallel over frames: 8 cores x 2 frames each. No collectives.

v2 schedule (vs the 477us baseline):
  - Attention is ACT(exp)-bound: 192 exps x ~1.1us = 213us/core. All other
    engines' work is arranged to hide under it.
  - Frame 1's qkv+rope is EMITTED interleaved with frame 0's attention
    units, with its own PSUM tag ring, so PE fills exp-wait gaps (the
    baseline serialized qkv(f1) after attention(f0) via the shared "mm"
    psum ring).
  - Softmax normalize: reciprocal straight off the PSUM denominator row,
    GPSIMD partition_broadcast, one DVE multiply (3 ops, was 6 + a PE
    broadcast matmul). Frees ~85us of DVE and ~10us of PE.
  - qkv psum->sbuf casts for frame 0 run on the idle Scalar engine.
  - Const/x DMAs issue from two queues (sync + scalar) with x chunk 0 and
    wqkv first: first matmul at ~10us instead of 26.5us.
  - Out-proj interleaves under frame 1's attention; out DMAs issue from
    the gpsimd queue.

Per-core layout (hardcoded): xT [768, 2048] d-major bf16; q,k TRANSPOSED
[64hd, seq] per head-pair so attention needs no transposes; v natural
[seq, 64+1] with a ones column emitting softmax denominators from the
same attn@v stream; scores transposed [keys, queries], exp on ACT with
the 1/8 scale folded in; softmax skips max-subtraction (scores are O(1),
exp exact to 2ULP).
"""
import sys

sys.path.insert(0, "/opt/trn_rl_repo")

import numpy as np
import ml_dtypes

BF16 = ml_dtypes.bfloat16

B, T, D = 2, 8, 768
NH, HD = 12, 64
NCORES = 8
FPC = 2  # frames per core

_built = {}

import os
F_NO_INTERLEAVE = os.environ.get("KV2_NO_INTERLEAVE", "0") == "1"
F_OLD_NORM = os.environ.get("KV2_OLD_NORM", "0") == "1"
F_NO_ACT_CAST = os.environ.get("KV2_NO_ACT_CAST", "0") == "1"
F_SYNC_DMA = os.environ.get("KV2_SYNC_DMA", "0") == "1"



def _host_rope(H, W, head_dim):
    """Replicates reference._rope_cos_sin in numpy fp32."""
    half = head_dim // 4
    inv_freq = (1.0 / (10000.0 ** (np.arange(half, dtype=np.float32) / half))).astype(
        np.float32
    )
    th_h = np.arange(H, dtype=np.float32)[:, None] * inv_freq  # [H, half]
    th_w = np.arange(W, dtype=np.float32)[:, None] * inv_freq  # [W, half]
    cos = np.concatenate(
        [
            np.broadcast_to(np.cos(th_h)[:, None, :], (H, W, half)),
            np.broadcast_to(np.cos(th_w)[None, :, :], (H, W, half)),
        ],
        axis=-1,
    )
    sin = np.concatenate(
        [
            np.broadcast_to(np.sin(th_h)[:, None, :], (H, W, half)),
            np.broadcast_to(np.sin(th_w)[None, :, :], (H, W, half)),
        ],
        axis=-1,
    )
    cos = np.repeat(cos, 2, axis=-1).reshape(H * W, head_dim).astype(np.float32)
    sin = np.repeat(sin, 2, axis=-1).reshape(H * W, head_dim).astype(np.float32)
    return cos, sin


def _rot_matT():
    """RT = R.T where (R @ q)[2i] = -q[2i+1], (R @ q)[2i+1] = q[2i]."""
    RT = np.zeros((128, 128), dtype=np.float32)
    for i in range(64):
        RT[2 * i + 1, 2 * i] = -1.0
        RT[2 * i, 2 * i + 1] = 1.0
    return RT


def build_nc(H, W):
    """Builds the per-core Bass program. S = H*W tokens per frame."""
    import concourse.bass as bass
    import concourse.tile as tile
    from concourse import bacc, mybir, library_config

    dt = mybir.dt
    S = H * W
    SL = FPC * S  # tokens per core
    QCS = min(512, S)  # query-chunk size
    NQC = S // QCS  # query chunks per frame
    KT = S // 128  # key tiles per frame
    SC = min(512, S)  # token chunk for qkv proj
    NSCF = S // SC  # qkv token chunks per frame
    NSTF = S // 128  # s-tiles per frame
    NDC = D // 128  # 6 d-chunks
    NPAIR = NH // 2  # 6 head pairs
    SPW = 2 * QCS  # scores tile width: 2 heads interleaved

    nc = bacc.Bacc("TRN2", target_bir_lowering=False, debug=False)

    xT = nc.dram_tensor("xT", [D, SL], dt.bfloat16, kind="ExternalInput")
    wqkvT = nc.dram_tensor("wqkvT", [D, 3 * D], dt.bfloat16, kind="ExternalInput")
    w_outT = nc.dram_tensor("w_outT", [D, D], dt.bfloat16, kind="ExternalInput")
    cosP = nc.dram_tensor("cosP", [128, SL], dt.bfloat16, kind="ExternalInput")
    sinP = nc.dram_tensor("sinP", [128, SL], dt.bfloat16, kind="ExternalInput")
    rotT = nc.dram_tensor("rotT", [128, 128], dt.bfloat16, kind="ExternalInput")
    bias_rep = nc.dram_tensor("bias_rep", [128, D], dt.float32, kind="ExternalInput")
    out = nc.dram_tensor("out", [SL, D], dt.float32, kind="ExternalOutput")

    ActF = mybir.ActivationFunctionType
    scale = 1.0 / np.sqrt(HD)

    with tile.TileContext(nc) as tc:
        import contextlib

        ctx = contextlib.ExitStack()
        with ctx:
            const = ctx.enter_context(tc.tile_pool(name="const", bufs=1))
            xt_pool = ctx.enter_context(tc.tile_pool(name="xt", bufs=14))
            qk_pool = ctx.enter_context(tc.tile_pool(name="qk", bufs=1))
            v_pool = ctx.enter_context(tc.tile_pool(name="v", bufs=1))
            ot_pool = ctx.enter_context(tc.tile_pool(name="ot", bufs=1))
            et_pool = ctx.enter_context(tc.tile_pool(name="et", bufs=7))
            rt_pool = ctx.enter_context(tc.tile_pool(name="rt", bufs=3))
            rc_pool = ctx.enter_context(tc.tile_pool(name="rc", bufs=3))
            bct_pool = ctx.enter_context(tc.tile_pool(name="bct", bufs=3))
            outsb_pool = ctx.enter_context(tc.tile_pool(name="outsb", bufs=2))
            sc_ps = ctx.enter_context(tc.tile_pool(name="scps", bufs=2, space="PSUM"))
            av_ps = ctx.enter_context(tc.tile_pool(name="avps", bufs=2, space="PSUM"))
            mm_ps = ctx.enter_context(tc.tile_pool(name="mmps", bufs=2, space="PSUM"))

            # ---- constants + x prefetch (two issue queues, critical first) ----
            xts = [[None] * NDC for _ in range(FPC * NSCF)]
            for gc in range(FPC * NSCF):
                for d in range(NDC):
                    xts[gc][d] = xt_pool.tile(
                        [128, SC], dt.bfloat16, tag="xt", name=f"xt_{gc}_{d}"
                    )
            w_t = []
            for d in range(NDC):
                w_t.append(
                    const.tile([128, 3 * D], dt.bfloat16, tag=f"wqkv{d}", name=f"wqkv{d}")
                )
            wo_t = []
            bias_t = None
            # sync queue: x chunk 0, then wqkv, then the rest of x
            for d in range(NDC):
                nc.sync.dma_start(xts[0][d][:], xT[d * 128 : (d + 1) * 128, 0:SC])
            for d in range(NDC):
                weng = nc.gpsimd if d % 2 == 0 else nc.scalar
                weng.dma_start(
                    w_t[d][:, 0 : 2 * D], wqkvT[d * 128 : (d + 1) * 128, 0 : 2 * D]
                )
            for d in range(NDC):
                weng = nc.gpsimd if d % 2 == 0 else nc.scalar
                weng.dma_start(
                    w_t[d][:, 2 * D : 3 * D], wqkvT[d * 128 : (d + 1) * 128, 2 * D : 3 * D]
                )
            for gc in range(1, FPC * NSCF):
                for d in range(NDC):
                    nc.sync.dma_start(
                        xts[gc][d][:], xT[d * 128 : (d + 1) * 128, gc * SC : (gc + 1) * SC]
                    )
            # scalar queue: rope tables, then out-proj weights (needed late)
            eng2 = nc.sync if F_SYNC_DMA else nc.scalar
            cos_t = const.tile([128, SL], dt.bfloat16, tag="cos")
            eng2.dma_start(cos_t[:], cosP[:])
            sin_t = const.tile([128, SL], dt.bfloat16, tag="sin")
            eng2.dma_start(sin_t[:], sinP[:])
            rot_t = const.tile([128, 128], dt.bfloat16, tag="rot")
            eng2.dma_start(rot_t[:], rotT[:])
            for d in range(NDC):
                t = const.tile([128, D], dt.bfloat16, tag=f"wout{d}", name=f"wout{d}")
                eng2.dma_start(t[:], w_outT[d * 128 : (d + 1) * 128, :])
                wo_t.append(t)
            bias_t = const.tile([128, D], dt.float32, tag="bias")
            eng2.dma_start(bias_t[:], bias_rep[:])
            ones_t = const.tile([1, 64], dt.bfloat16, tag="ones")
            nc.vector.memset(ones_t[:], 1.0)
            nc.gpsimd.load_library(library_config.attn)

            # ---- per-frame activation tiles ----
            frames = {}

            def make_frame(f):
                fr = {}
                fr["qk_q"] = [
                    qk_pool.tile([128, S], dt.bfloat16, tag=f"q{p}", bufs=2, name=f"qkq{f}_{p}")
                    for p in range(NPAIR)
                ]
                fr["qk_k"] = [
                    qk_pool.tile([128, S], dt.bfloat16, tag=f"k{p}", bufs=2, name=f"qkk{f}_{p}")
                    for p in range(NPAIR)
                ]
                fr["v_sb"] = [
                    v_pool.tile([128, NH * 65], dt.bfloat16, tag=f"v{i}", bufs=2, name=f"vsb{f}_{i}")
                    for i in range(NSTF)
                ]
                for i in range(NSTF):
                    vv = fr["v_sb"][i][:].rearrange("p (h c) -> p h c", h=NH)
                    nc.vector.memset(vv[:, :, 64:65], 1.0)
                frames[f] = fr

            def make_frame_ot(f):
                frames[f]["ot_sb"] = [
                    ot_pool.tile([128, S], dt.bfloat16, tag=f"ot{d}", bufs=2, name=f"otsb{f}_{d}")
                    for d in range(NDC)
                ]

            # ---- emit helpers ----
            def emit_qk_chain(f, c, et):
                """One [128e, 512tok] output chain of the transposed q/k proj."""
                gc = f * NSCF + c
                ps = mm_ps.tile([128, SC], dt.float32, tag="mm", name=f"qkps_{f}_{c}_{et}")
                for d in range(NDC):
                    nc.tensor.matmul(
                        ps[:],
                        w_t[d][:, et * 128 : (et + 1) * 128],
                        xts[gc][d][:],
                        start=(d == 0),
                        stop=(d == NDC - 1),
                    )
                fr = frames[f]
                dst = fr["qk_q"][et] if et < NDC else fr["qk_k"][et - NDC]
                eng = nc.scalar if (f == 0 and not F_NO_ACT_CAST) else nc.vector
                if eng is nc.scalar:
                    nc.scalar.copy(dst[:, c * SC : (c + 1) * SC], ps[:])
                else:
                    nc.vector.tensor_copy(dst[:, c * SC : (c + 1) * SC], ps[:])

            def emit_rope(f, p, which, c):
                """RoPE chunk c for q or k of pair p: tens = tens*cos + rot(tens)*sin."""
                fr = frames[f]
                tens = fr["qk_q"][p] if which == "q" else fr["qk_k"][p]
                sl_ = slice(c * SC, (c + 1) * SC)
                rps = mm_ps.tile([128, SC], dt.float32, tag="mm", name=f"rps_{f}_{p}_{which}_{c}")
                nc.tensor.matmul(rps[:], rot_t[:], tens[:, sl_], start=True, stop=True)
                t1 = rt_pool.tile([128, SC], dt.bfloat16, tag="rt1", name=f"rt1_{f}_{p}_{which}_{c}")
                nc.vector.tensor_mul(t1[:], rps[:], sin_t[:, sl_])
                t2 = rt_pool.tile([128, SC], dt.bfloat16, tag="rt2", name=f"rt2_{f}_{p}_{which}_{c}")
                nc.vector.tensor_mul(t2[:], tens[:, sl_], cos_t[:, sl_])
                nc.vector.tensor_add(tens[:, sl_], t1[:], t2[:])

            def emit_v_chain(f, c, st, nch):
                """V projection for s-tile (c,st), e-columns [n0:n1)."""
                gc = f * NSCF + c
                lst = c * (SC // 128) + st
                n0, n1 = (0, 512) if nch == 0 else (512, D)
                h0, h1 = (0, 8) if nch == 0 else (8, NH)
                ps = mm_ps.tile([128, SC], dt.float32, tag="mm", name=f"vps_{f}_{lst}_{nch}")
                for d in range(NDC):
                    nc.tensor.matmul(
                        ps[:, : n1 - n0],
                        xts[gc][d][:, st * 128 : (st + 1) * 128],
                        w_t[d][:, 2 * D + n0 : 2 * D + n1],
                        start=(d == 0),
                        stop=(d == NDC - 1),
                    )
                vv = frames[f]["v_sb"][lst][:].rearrange("p (h c) -> p h c", h=NH)
                pv = ps[:, : n1 - n0].rearrange("p (h c) -> p h c", c=HD)
                if f == 0 and not F_NO_ACT_CAST:
                    nc.scalar.copy(vv[:, h0:h1, 0:HD], pv[:])
                else:
                    nc.vector.tensor_copy(vv[:, h0:h1, 0:HD], pv[:])

            def emit_att_unit(f, p, qc):
                """Attention for head-pair p, query chunk qc: scores+exp+av+norm."""
                fr = frames[f]
                qsl = slice(qc * QCS, (qc + 1) * QCS)
                avp = [
                    av_ps.tile([128, QCS], dt.float32, tag="av", name=f"avp_{f}_{p}_{qc}_{hh}")
                    for hh in range(2)
                ]
                LAG = 4
                ets = [None] * KT

                def emit_av(g):
                    for hh in range(2):
                        h = 2 * p + hh
                        nc.tensor.matmul(
                            avp[hh][0:65, :],
                            fr["v_sb"][g][:, h * 65 : h * 65 + 65],
                            ets[g][:, hh * QCS : (hh + 1) * QCS],
                            start=(g == 0),
                            stop=(g == KT - 1),
                        )

                for g in range(KT):
                    ksl = slice(g * 128, (g + 1) * 128)
                    sp = sc_ps.tile([128, SPW], dt.float32, tag="sc", name=f"sp_{f}_{p}_{qc}_{g}")
                    for hh in range(2):
                        rb = 64 * hh
                        nc.tensor.matmul(
                            sp[:, hh * QCS : (hh + 1) * QCS],
                            fr["qk_k"][p][rb : rb + 64, ksl],
                            fr["qk_q"][p][rb : rb + 64, qsl],
                            start=True,
                            stop=True,
                            tile_position=(rb, 0),
                        )
                    ets[g] = et_pool.tile([128, SPW], dt.bfloat16, tag="et", name=f"et_{f}_{p}_{qc}_{g}")
                    nc.scalar.activation(ets[g][:], sp[:], ActF.Exp, scale=float(scale))
                    if g >= LAG:
                        emit_av(g - LAG)
                for g in range(KT - LAG, KT):
                    emit_av(g)
                # normalize: o[0:64] * (1/r), r = row 64 of the psum accumulator
                for hh in range(2):
                    rb = 64 * hh
                    if F_OLD_NORM:
                        rr = rc_pool.tile([1, QCS], dt.float32, tag="rr", name=f"rr_{f}_{p}_{qc}_{hh}")
                        nc.vector.tensor_copy(rr[:], avp[hh][64:65, :])
                        ou = bct_pool.tile([64, QCS], dt.float32, tag="ou", name=f"ou_{f}_{p}_{qc}_{hh}")
                        nc.vector.tensor_copy(ou[:], avp[hh][0:64, :])
                        rc = rc_pool.tile([1, QCS], dt.float32, tag="rc", name=f"rc_{f}_{p}_{qc}_{hh}")
                        nc.vector.reciprocal_approx_fast(rc[:], rr[:])
                        rcb = rc_pool.tile([1, QCS], dt.bfloat16, tag="rcb", name=f"rcb_{f}_{p}_{qc}_{hh}")
                        nc.vector.tensor_copy(rcb[:], rc[:])
                        bc = mm_ps.tile([128, QCS], dt.float32, tag="mm", name=f"bc_{f}_{p}_{qc}_{hh}")
                        nc.tensor.matmul(bc[0:64, :], ones_t[:], rcb[:], start=True, stop=True)
                        nc.vector.tensor_mul(
                            fr["ot_sb"][p][rb : rb + 64, qsl], ou[:], bc[0:64, :]
                        )
                    else:
                        rr = rc_pool.tile([1, QCS], dt.float32, tag="rr", name=f"nrr_{f}_{p}_{qc}_{hh}")
                        nc.vector.tensor_copy(rr[:], avp[hh][64:65, :])
                        rc = rc_pool.tile([1, QCS], dt.float32, tag="rc", name=f"rc_{f}_{p}_{qc}_{hh}")
                        nc.vector.reciprocal_approx_fast(rc[:], rr[:])
                        bct = bct_pool.tile([64, QCS], dt.float32, tag="bct", name=f"bct_{f}_{p}_{qc}_{hh}")
                        nc.gpsimd.partition_broadcast(bct[:], rc[:])
                        nc.vector.tensor_mul(
                            fr["ot_sb"][p][rb : rb + 64, qsl], avp[hh][0:64, :], bct[:]
                        )

            def emit_outproj(f, st):
                fr = frames[f]
                osb = outsb_pool.tile([128, D], dt.float32, tag="osb", name=f"osb_{f}_{st}")
                for nch in range(2):
                    n0, n1 = (0, 512) if nch == 0 else (512, D)
                    ps = mm_ps.tile([128, 512], dt.float32, tag="mm", name=f"ops_{f}_{st}_{nch}")
                    for d in range(NDC):
                        nc.tensor.matmul(
                            ps[:, : n1 - n0],
                            fr["ot_sb"][d][:, st * 128 : (st + 1) * 128],
                            wo_t[d][:, n0:n1],
                            start=(d == 0),
                            stop=(d == NDC - 1),
                        )
                    nc.vector.tensor_add(osb[:, n0:n1], ps[:, : n1 - n0], bias_t[:, n0:n1])
                (nc.sync if F_SYNC_DMA else nc.gpsimd).dma_start(
                    out[f * S + st * 128 : f * S + (st + 1) * 128, :], osb[:]
                )

            def qkv_items(f):
                """Closures producing frame f's qkv+rope in dependency-friendly
                order: per chunk, k-projections (then their rope) before q."""
                items = []
                for c in range(NSCF):
                    for et in list(range(NDC, 2 * NDC)) + list(range(NDC)):
                        p = et - NDC if et >= NDC else et
                        which = "k" if et >= NDC else "q"
                        items.append(
                            lambda f=f, c=c, et=et, p=p, w=which: (
                                emit_qk_chain(f, c, et),
                                emit_rope(f, p, w, c),
                            )
                        )
                    for st in range(SC // 128):
                        for nch in range(2):
                            items.append(
                                lambda f=f, c=c, st=st, nch=nch: emit_v_chain(f, c, st, nch)
                            )
                return items

            units = [(p, qc) for p in range(NPAIR) for qc in range(NQC)]

            # ---- P1: frame 0 qkv + rope (casts/copies on idle ACT) ----
            make_frame(0)
            for it in qkv_items(0):
                it()

            # ---- P2: frame 0 attention interleaved with frame 1 qkv ----
            make_frame(1)
            make_frame_ot(0)
            work = qkv_items(1)
            done = 0
            for i, (p, qc) in enumerate(units):
                emit_att_unit(0, p, qc)
                if F_NO_INTERLEAVE:
                    continue
                want = (i + 1) * len(work) // len(units)
                while done < want:
                    work[done]()
                    done += 1
            while done < len(work):
                work[done]()
                done += 1

            # ---- P3: frame 1 attention interleaved with frame 0 out-proj ----
            make_frame_ot(1)
            done = 0
            for i, (p, qc) in enumerate(units):
                emit_att_unit(1, p, qc)
                if F_NO_INTERLEAVE:
                    continue
                want = (i + 1) * NSTF // len(units)
                while done < want:
                    emit_outproj(0, done)
                    done += 1
            while done < NSTF:
                emit_outproj(0, done)
                done += 1

            # ---- P4: frame 1 out-proj ----
            for st in range(NSTF):
                emit_outproj(1, st)

    nc.compile()
    return nc


def _prep_inputs(x, w_qkv, w_out, b_out, H, W):
    """Host-side prep: shard + transpose + cast. Returns per-core in_maps."""
    S = H * W
    SL = FPC * S
    nframes = x.shape[0] * x.shape[1]
    ncores = nframes // FPC
    xf = np.asarray(x, dtype=np.float32).reshape(nframes, S, D)

    wqkvT = np.ascontiguousarray(np.asarray(w_qkv, np.float32).T).astype(BF16)
    w_outT = np.ascontiguousarray(np.asarray(w_out, np.float32).T).astype(BF16)
    cos, sin = _host_rope(H, W, HD)  # [S, 64]
    cosP = np.tile(cos.T, (2, FPC)).astype(BF16)  # [128, SL]
    sinP = np.tile(sin.T, (2, FPC)).astype(BF16)
    rotT = _rot_matT().astype(BF16)
    bias_rep = np.tile(np.asarray(b_out, np.float32)[None, :], (128, 1))

    in_maps = []
    for c in range(ncores):
        shard = xf[c * FPC : (c + 1) * FPC].reshape(SL, D)
        xT = np.ascontiguousarray(shard.T).astype(BF16)  # [768, SL]
        in_maps.append(
            dict(
                xT=xT,
                wqkvT=wqkvT,
                w_outT=w_outT,
                cosP=cosP,
                sinP=sinP,
                rotT=rotT,
                bias_rep=bias_rep,
            )
        )
    return in_maps


def run(x, w_qkv, w_out, b_out, trace=False, tmpdir=None):
    from concourse import bass_utils

    Hd, Wd = x.shape[2], x.shape[3]
    key = (Hd, Wd)
    if key not in _built:
        _built[key] = build_nc(Hd, Wd)
    nc = _built[key]
    in_maps = _prep_inputs(x, w_qkv, w_out, b_out, Hd, Wd)
    res = bass_utils.run_bass_kernel_spmd(
        nc, in_maps, core_ids=list(range(len(in_maps))), trace=trace, tmpdir=tmpdir
    )
    S = Hd * Wd
    outs = [r["out"] for r in res.results]
    full = np.concatenate(outs, axis=0).reshape(B, T, Hd, Wd, D).astype(np.float32)
    return full, res


def kernel(x, w_qkv, w_out, b_out):
    full, _ = run(x, w_qkv, w_out, b_out, trace=False)
    return full
